# revision 4
# baseline (speedup 1.0000x reference)
"""Conv2d 3x3 (N=32, C_in=128, H=W=56, C_out=256, stride 1, pad 1) on 8 TRN2
NeuronCores — bf16 implicit-GEMM at the PE fill rate.

Data-parallel over batch (4 images per core).  Per core: implicit GEMM with
C_in=128 on partitions; 9 taps x 7 h-tiles x 2 C_out-chunks x 4 images =
504 bf16 matmuls (448 free dim) accumulating in PSUM.

Design points, each measured on this hardware (448-free LDW+MM streams):
- bf16 matmuls with a per-tile ACT drain in the stream run at the pure
  fill rate ~183-187 ns/MM; f32r self-loading runs at 247 ns/MM.
- rhs windows must keep a 56-wide contiguous inner dim: x is stored
  column-padded [56 rows, 58 cols] per image (side columns zeroed once),
  vertical taps are row-clipped, horizontal taps use the zero columns.
- x tiles must be persistent bufs=1 tiles: sourcing them from a multi-
  buffer tile pool degrades the whole MM stream 185 -> 225 ns/MM.
- In-loop reloads are hidden by manual ping-pong: two persistent tile
  sets; each loop body runs [load B | compute A | load A | compute B],
  so every DMA lands one full conv ahead of its readers.
- ACT drains fuse +bias and the bf16 downcast; the host upcasts.
"""

import os

import numpy as np

N, C_IN, H, W = 32, 128, 56, 56
C_OUT, KH, KW = 256, 3, 3
NCORES = 8
NIMG = N // NCORES
P = 128
NCHUNK = C_OUT // P
KHW = KH * KW
HT = 8
NT = H // HT
FREE = HT * W
WP = W + 2
NWU = 20

TAPS = [(1, 1), (0, 0), (0, 1), (0, 2), (1, 0), (1, 2), (2, 0), (2, 1), (2, 2)]

_CACHE = {}


def _build(repeat: int = 1):
    import concourse.tile as tile
    from concourse import bacc, mybir

    f32 = mybir.dt.float32
    bf16 = mybir.dt.bfloat16

    imgmajor = os.environ.get("K7_IMGMAJOR", "0") == "1"
    ob_bufs = int(os.environ.get("K7_OB", "3"))
    ps_bufs = int(os.environ.get("K7_PS", "8"))
    xchunks = int(os.environ.get("K7_XCHUNKS", "2"))
    out_gran = os.environ.get("K7_OUT_GRAN", "half")  # half | tile | plane
    out_eng = os.environ.get("K7_OUT_ENG", "scalar")  # scalar | sync | pool
    in_eng = os.environ.get("K7_IN_ENG", "sync")      # sync | scalar | pool
    noload = os.environ.get("K7_NOLOAD", "0") == "1"
    nostore = os.environ.get("K7_NOSTORE", "0") == "1"
    nwu = int(os.environ.get("K7_NWU", str(NWU)))
    nobias = os.environ.get("K7_NOBIAS", "0") == "1"
    onec = os.environ.get("K7_ONEC", "0") == "1"
    oneimg = os.environ.get("K7_ONEIMG", "0") == "1"

    assert repeat == 1 or repeat % 2 == 0, "repeat must be 1 or even"

    nc = bacc.Bacc("TRN2", target_bir_lowering=False, debug=False)

    x_d = nc.dram_tensor("x", [NIMG, P, H * W], bf16, kind="ExternalInput").ap()
    w_d = nc.dram_tensor("w", [P, KHW, NCHUNK, P], bf16, kind="ExternalInput").ap()
    b_d = nc.dram_tensor("b", [P, NCHUNK], f32, kind="ExternalInput").ap()
    out_flat = nc.dram_tensor(
        "out", [NIMG, NCHUNK, P, H * W], bf16, kind="ExternalOutput"
    ).ap()

    eng = {"sync": nc.sync, "scalar": nc.scalar, "pool": nc.gpsimd}
    in_dma = eng[in_eng].dma_start
    out_dma = eng[out_eng].dma_start

    nsets = 1 if repeat == 1 else 2

    with tile.TileContext(nc) as tc:
        with (
            tc.tile_pool(name="wpool", bufs=1) as wpool,
            tc.tile_pool(name="pspool", bufs=ps_bufs, space="PSUM") as pspool,
            tc.tile_pool(name="obpool", bufs=ob_bufs) as obpool,
        ):
            # PE warmup vs the HAM clock gate
            wu = wpool.tile([P, 256], bf16, tag="wu")
            nc.vector.memset(wu[:], 0.5)
            pswu = pspool.tile([P, FREE], f32, tag="ps", name="wu")
            for _ in range(nwu):
                nc.tensor.matmul(pswu[:, 0:256], wu[:, 0:P], wu[:], start=True, stop=True)

            # weights chunk 0 first (critical path)
            wt = wpool.tile([P, KHW, NCHUNK, P], bf16, tag="wt")
            nc.scalar.dma_start(wt[:, :, 0, :], w_d[:, :, 0, :])

            # persistent column-padded x tiles; the zero side columns are
            # baked into the DRAM layout host-side, so the loads are plain
            # contiguous row transfers and no engine ever touches borders
            # (a DVE-produced border would hang DVE-sem waits on the PE
            # matmul stream across staggered-reset iterations).
            sets = []
            for s in range(nsets):
                xis = [
                    wpool.tile([P, H * W], bf16, tag=f"xi{s}_{i}", name=f"xi{s}_{i}")
                    for i in range(NIMG)
                ]
                xi3s = [xi[:].rearrange("p (h w) -> p h w", w=W) for xi in xis]
                sets.append(xi3s)

            def emit_load_img(xi3, img, nchunks=xchunks):
                rows = H // nchunks
                for j in range(nchunks):
                    in_dma(
                        xi3[:, j * rows : (j + 1) * rows, :],
                        x_d[img, :, j * rows * W : (j + 1) * rows * W],
                    )

            def emit_load_set(xi3s):
                for img in range(NIMG):
                    emit_load_img(xi3s[img], img)

            # initial load of set 0, then c1 weights + bias
            emit_load_img(sets[0][0], 0)
            nc.scalar.dma_start(wt[:, :, 1, :], w_d[:, :, 1, :])
            for img in range(1, NIMG):
                emit_load_img(sets[0][img], img)
            if noload and nsets > 1:
                emit_load_set(sets[1])
            bt = wpool.tile([P, NCHUNK], f32, tag="bt")
            nc.scalar.dma_start(bt[:], b_d[:])

            ROWT = [(t * HT, t * HT + HT) for t in range(NT)]

            def emit_compute_plane(s, img, c, xi3, last_plane=False, uid=None):
                uid = uid if uid is not None else f"{img}_{c}"
                chunked_out = last_plane or out_gran == "tile"
                ob = obpool.tile([P, H * W], bf16, tag="ob", name=f"ob{s}_{uid}")
                half_at = len(ROWT) // 2
                for ti, (r0, r1) in enumerate(ROWT):
                    ps = pspool.tile(
                        [P, FREE], f32, tag="ps", name=f"ps{s}_{uid}_{ti}",
                        bufs=ps_bufs,
                    )
                    ps3 = ps[:, 0 : (r1 - r0) * W].rearrange("p (h w) -> p h w", w=W)
                    for ki, (kh, kw) in enumerate(TAPS):
                        dh, dw = kh - 1, kw - 1
                        R0, R1 = max(r0, -dh), min(r1, H - dh)
                        C0, C1 = max(0, -dw), min(W, W - dw)
                        rhs = xi3[:, R0 + dh : R1 + dh, C0 + dw : C1 + dw]
                        out_ap = ps3[:, R0 - r0 : R1 - r0, C0:C1]
                        nc.tensor.matmul(
                            out_ap, wt[:, kh * KW + kw, c, :], rhs,
                            start=(ki == 0), stop=(ki == KHW - 1),
                        )
                    _bias_kw = {} if nobias else {"bias": bt[:, c : c + 1]}
                    nc.scalar.activation(
                        ob[:, r0 * W : r1 * W], ps[:, 0 : (r1 - r0) * W],
                        mybir.ActivationFunctionType.Identity,
                        **_bias_kw,
                    )
                    if nostore:
                        continue
                    if chunked_out:
                        out_dma(
                            out_flat[img, c, :, r0 * W : r1 * W],
                            ob[:, r0 * W : r1 * W],
                        )
                    elif out_gran == "half" and ti == half_at - 1:
                        out_dma(out_flat[img, c, :, 0 : r1 * W], ob[:, 0 : r1 * W])
                if nostore:
                    if last_plane:
                        out_dma(out_flat[img, c, :, 0:FREE], ob[:, 0:FREE])
                    return
                if not chunked_out:
                    if out_gran == "half":
                        r_half = ROWT[half_at - 1][1]
                        out_dma(
                            out_flat[img, c, :, r_half * W : H * W],
                            ob[:, r_half * W : H * W],
                        )
                    else:
                        out_dma(out_flat[img, c], ob[:])

            def emit_compute_set(s):
                xi3s = sets[s]
                order = (
                    [(img, c) for img in range(NIMG) for c in range(NCHUNK)]
                    if imgmajor
                    else [(img, c) for c in range(NCHUNK) for img in range(NIMG)]
                )
                for n, (img, c) in enumerate(order):
                    emit_compute_plane(
                        s, img, 0 if onec else c,
                        xi3s[0 if oneimg else img],
                        last_plane=(n == len(order) - 1), uid=str(n),
                    )

            if repeat == 1:
                emit_compute_set(0)
            else:
                # ping-pong: each body does two full convs; every load set
                # lands one conv ahead of its readers.
                with tc.For_i(
                    0, repeat // 2, 1,
                    staggered_reset=True,
                    hint_engines=(
                        mybir.EngineType.PE,
                        mybir.EngineType.SP,
                        mybir.EngineType.Activation,
                    ),
                ):
                    if not noload:
                        emit_load_set(sets[1])
                    emit_compute_set(0)
                    if not noload:
                        emit_load_set(sets[0])
                    emit_compute_set(1)

    nc.compile()
    return nc


def make_in_maps(x, weight, bias):
    import ml_dtypes

    bf16 = ml_dtypes.bfloat16
    x_t = np.ascontiguousarray(
        x.astype(bf16).reshape(NCORES, NIMG, P, H * W)
    )
    w_t = np.ascontiguousarray(
        weight.astype(np.float32).transpose(1, 2, 3, 0).reshape(P, KHW, NCHUNK, P).astype(bf16)
    )
    b_t = np.ascontiguousarray(bias.astype(np.float32).reshape(NCHUNK, P).T)
    return [{"x": x_t[i], "w": w_t, "b": b_t} for i in range(NCORES)]


def kernel(x: np.ndarray, weight: np.ndarray, bias: np.ndarray) -> np.ndarray:
    from concourse.bass_utils import run_bass_kernel_spmd

    if "nc" not in _CACHE:
        _CACHE["nc"] = _build()
    nc = _CACHE["nc"]

    in_maps = make_in_maps(x, weight, bias)
    res = run_bass_kernel_spmd(nc, in_maps, list(range(NCORES)))
    out = np.concatenate(
        [
            r["out"].astype(np.float32).reshape(NIMG, C_OUT, H, W)
            for r in res.results
        ],
        axis=0,
    )
    return out


# revision 5
# speedup vs baseline: 1.0070x; 1.0070x over previous
"""Conv2d 3x3 (N=32, C_in=128, H=W=56, C_out=256, stride 1, pad 1) on 8 TRN2
NeuronCores — bf16 implicit-GEMM at the PE fill rate.

Data-parallel over batch (4 images per core).  Per core: implicit GEMM with
C_in=128 on partitions; 9 taps x 7 h-tiles x 2 C_out-chunks x 4 images =
504 bf16 matmuls (448 free dim) accumulating in PSUM.

Design points, each measured on this hardware (448-free LDW+MM streams):
- bf16 matmuls with a per-tile ACT drain in the stream run at the pure
  fill rate ~183-187 ns/MM; f32r self-loading runs at 247 ns/MM.
- rhs windows must keep a 56-wide contiguous inner dim: x is stored
  column-padded [56 rows, 58 cols] per image (side columns zeroed once),
  vertical taps are row-clipped, horizontal taps use the zero columns.
- x tiles must be persistent bufs=1 tiles: sourcing them from a multi-
  buffer tile pool degrades the whole MM stream 185 -> 225 ns/MM.
- In-loop reloads are hidden by manual ping-pong: two persistent tile
  sets; each loop body runs [load B | compute A | load A | compute B],
  so every DMA lands one full conv ahead of its readers.
- ACT drains fuse +bias and the bf16 downcast; the host upcasts.
"""

import os

import numpy as np

N, C_IN, H, W = 32, 128, 56, 56
C_OUT, KH, KW = 256, 3, 3
NCORES = 8
NIMG = N // NCORES
P = 128
NCHUNK = C_OUT // P
KHW = KH * KW
HT = 8
NT = H // HT
FREE = HT * W
WP = W + 2
NWU = 20

TAPS = [(1, 1), (0, 0), (0, 1), (0, 2), (1, 0), (1, 2), (2, 0), (2, 1), (2, 2)]

_CACHE = {}


def _build(repeat: int = 1):
    import concourse.tile as tile
    from concourse import bacc, mybir

    f32 = mybir.dt.float32
    bf16 = mybir.dt.bfloat16

    imgmajor = os.environ.get("K7_IMGMAJOR", "0") == "1"
    ob_bufs = int(os.environ.get("K7_OB", "3"))
    ps_bufs = int(os.environ.get("K7_PS", "8"))
    xchunks = int(os.environ.get("K7_XCHUNKS", "2"))
    out_gran = os.environ.get("K7_OUT_GRAN", "half")  # half | tile | plane
    out_eng = os.environ.get("K7_OUT_ENG", "scalar")  # scalar | sync | pool
    in_eng = os.environ.get("K7_IN_ENG", "sync")      # sync | scalar | pool
    noload = os.environ.get("K7_NOLOAD", "0") == "1"
    nostore = os.environ.get("K7_NOSTORE", "0") == "1"
    nwu = int(os.environ.get("K7_NWU", str(NWU)))
    nobias = os.environ.get("K7_NOBIAS", "0") == "1"
    onec = os.environ.get("K7_ONEC", "0") == "1"
    oneimg = os.environ.get("K7_ONEIMG", "0") == "1"
    single = os.environ.get("K7_SINGLE", "0") == "1"  # one conv per loop body

    assert repeat == 1 or repeat % 2 == 0, "repeat must be 1 or even"

    nc = bacc.Bacc("TRN2", target_bir_lowering=False, debug=False)

    x_d = nc.dram_tensor("x", [NIMG, P, H * W], bf16, kind="ExternalInput").ap()
    w_d = nc.dram_tensor("w", [P, NCHUNK, KHW, P], bf16, kind="ExternalInput").ap()
    b_d = nc.dram_tensor("b", [P, NCHUNK], f32, kind="ExternalInput").ap()
    out_flat = nc.dram_tensor(
        "out", [NIMG, NCHUNK, P, H * W], bf16, kind="ExternalOutput"
    ).ap()

    eng = {"sync": nc.sync, "scalar": nc.scalar, "pool": nc.gpsimd}
    in_dma = eng[in_eng].dma_start
    out_dma = eng[out_eng].dma_start

    nsets = 1 if (repeat == 1 or single) else 2

    with tile.TileContext(nc) as tc:
        with (
            tc.tile_pool(name="wpool", bufs=1) as wpool,
            tc.tile_pool(name="pspool", bufs=ps_bufs, space="PSUM") as pspool,
            tc.tile_pool(name="obpool", bufs=ob_bufs) as obpool,
        ):
            # PE warmup vs the HAM clock gate
            wu = wpool.tile([P, 256], bf16, tag="wu")
            nc.vector.memset(wu[:], 0.5)
            pswu = pspool.tile([P, FREE], f32, tag="ps", name="wu")
            for _ in range(nwu):
                nc.tensor.matmul(pswu[:, 0:256], wu[:, 0:P], wu[:], start=True, stop=True)

            # weights chunk 0 first (critical path)
            wt = wpool.tile([P, NCHUNK, KHW, P], bf16, tag="wt")
            nc.scalar.dma_start(wt[:, 0, :, :], w_d[:, 0, :, :])

            # persistent column-padded x tiles; the zero side columns are
            # baked into the DRAM layout host-side, so the loads are plain
            # contiguous row transfers and no engine ever touches borders
            # (a DVE-produced border would hang DVE-sem waits on the PE
            # matmul stream across staggered-reset iterations).
            sets = []
            for s in range(nsets):
                xis = [
                    wpool.tile([P, H * W], bf16, tag=f"xi{s}_{i}", name=f"xi{s}_{i}")
                    for i in range(NIMG)
                ]
                xi3s = [xi[:].rearrange("p (h w) -> p h w", w=W) for xi in xis]
                sets.append(xi3s)

            def emit_load_img(xi3, img, nchunks=xchunks):
                rows = H // nchunks
                for j in range(nchunks):
                    in_dma(
                        xi3[:, j * rows : (j + 1) * rows, :],
                        x_d[img, :, j * rows * W : (j + 1) * rows * W],
                    )

            def emit_load_set(xi3s):
                for img in range(NIMG):
                    emit_load_img(xi3s[img], img)

            # initial load of set 0, then c1 weights + bias
            emit_load_img(sets[0][0], 0)
            nc.scalar.dma_start(wt[:, 1, :, :], w_d[:, 1, :, :])
            for img in range(1, NIMG):
                emit_load_img(sets[0][img], img)
            if noload and nsets > 1:
                emit_load_set(sets[1])
            bt = wpool.tile([P, NCHUNK], f32, tag="bt")
            nc.scalar.dma_start(bt[:], b_d[:])

            ROWT = [(t * HT, t * HT + HT) for t in range(NT)]

            def emit_compute_plane(s, img, c, xi3, last_plane=False, uid=None):
                uid = uid if uid is not None else f"{img}_{c}"
                chunked_out = last_plane or out_gran == "tile"
                ob = obpool.tile([P, H * W], bf16, tag="ob", name=f"ob{s}_{uid}")
                half_at = len(ROWT) // 2
                for ti, (r0, r1) in enumerate(ROWT):
                    ps = pspool.tile(
                        [P, FREE], f32, tag="ps", name=f"ps{s}_{uid}_{ti}",
                        bufs=ps_bufs,
                    )
                    ps3 = ps[:, 0 : (r1 - r0) * W].rearrange("p (h w) -> p h w", w=W)
                    for ki, (kh, kw) in enumerate(TAPS):
                        dh, dw = kh - 1, kw - 1
                        R0, R1 = max(r0, -dh), min(r1, H - dh)
                        C0, C1 = max(0, -dw), min(W, W - dw)
                        rhs = xi3[:, R0 + dh : R1 + dh, C0 + dw : C1 + dw]
                        out_ap = ps3[:, R0 - r0 : R1 - r0, C0:C1]
                        nc.tensor.matmul(
                            out_ap, wt[:, c, kh * KW + kw, :], rhs,
                            start=(ki == 0), stop=(ki == KHW - 1),
                        )
                    _bias_kw = {} if nobias else {"bias": bt[:, c : c + 1]}
                    nc.scalar.activation(
                        ob[:, r0 * W : r1 * W], ps[:, 0 : (r1 - r0) * W],
                        mybir.ActivationFunctionType.Identity,
                        **_bias_kw,
                    )
                    if nostore:
                        continue
                    if chunked_out:
                        out_dma(
                            out_flat[img, c, :, r0 * W : r1 * W],
                            ob[:, r0 * W : r1 * W],
                        )
                    elif out_gran == "half" and ti == half_at - 1:
                        out_dma(out_flat[img, c, :, 0 : r1 * W], ob[:, 0 : r1 * W])
                if nostore:
                    if last_plane:
                        out_dma(out_flat[img, c, :, 0:FREE], ob[:, 0:FREE])
                    return
                if not chunked_out:
                    if out_gran == "half":
                        r_half = ROWT[half_at - 1][1]
                        out_dma(
                            out_flat[img, c, :, r_half * W : H * W],
                            ob[:, r_half * W : H * W],
                        )
                    else:
                        out_dma(out_flat[img, c], ob[:])

            def emit_compute_set(s):
                xi3s = sets[s]
                order = (
                    [(img, c) for img in range(NIMG) for c in range(NCHUNK)]
                    if imgmajor
                    else [(img, c) for c in range(NCHUNK) for img in range(NIMG)]
                )
                for n, (img, c) in enumerate(order):
                    emit_compute_plane(
                        s, img, 0 if onec else c,
                        xi3s[0 if oneimg else img],
                        last_plane=(n == len(order) - 1), uid=str(n),
                    )

            if repeat == 1:
                emit_compute_set(0)
            elif single:
                emit_compute_set(0)  # prologue conv: hosts the staggered-
                # reset splice so the loop body stays steady-state clean
                with tc.For_i(
                    0, repeat, 1,
                    staggered_reset=True,
                    hint_engines=(
                        mybir.EngineType.PE,
                        mybir.EngineType.SP,
                        mybir.EngineType.Activation,
                    ),
                ):
                    if not noload:
                        emit_load_set(sets[0])
                    emit_compute_set(0)
            else:
                # ping-pong: each body does two full convs; every load set
                # lands one conv ahead of its readers.
                with tc.For_i(
                    0, repeat // 2, 1,
                    staggered_reset=True,
                    hint_engines=(
                        mybir.EngineType.PE,
                        mybir.EngineType.SP,
                        mybir.EngineType.Activation,
                    ),
                ):
                    if not noload:
                        emit_load_set(sets[1])
                    emit_compute_set(0)
                    if not noload:
                        emit_load_set(sets[0])
                    emit_compute_set(1)

    nc.compile()
    return nc


def make_in_maps(x, weight, bias):
    import ml_dtypes

    bf16 = ml_dtypes.bfloat16
    x_t = np.ascontiguousarray(
        x.astype(bf16).reshape(NCORES, NIMG, P, H * W)
    )
    w_t = np.ascontiguousarray(
        weight.astype(np.float32)
        .transpose(1, 2, 3, 0)
        .reshape(P, KHW, NCHUNK, P)
        .transpose(0, 2, 1, 3)
        .astype(bf16)
    )
    b_t = np.ascontiguousarray(bias.astype(np.float32).reshape(NCHUNK, P).T)
    return [{"x": x_t[i], "w": w_t, "b": b_t} for i in range(NCORES)]


def kernel(x: np.ndarray, weight: np.ndarray, bias: np.ndarray) -> np.ndarray:
    from concourse.bass_utils import run_bass_kernel_spmd

    if "nc" not in _CACHE:
        _CACHE["nc"] = _build()
    nc = _CACHE["nc"]

    in_maps = make_in_maps(x, weight, bias)
    res = run_bass_kernel_spmd(nc, in_maps, list(range(NCORES)))
    out = np.concatenate(
        [
            r["out"].astype(np.float32).reshape(NIMG, C_OUT, H, W)
            for r in res.results
        ],
        axis=0,
    )
    return out
